# revision 45
# baseline (speedup 1.0000x reference)
"""Trainium2 Bass kernel for nn_Block_15066745274698 (GQA attention block).

Computation (B=1, T=4096, C=2048, 16 heads x 128, 4 KV groups):
  qkv = x @ W_attn.T ; split q/k/v ; RoPE(q, k) ; causal GQA attention ;
  out = y @ W_proj.T

Sharding: head-parallel over 8 cores, 2 query heads + their KV group per
core. No collectives: each core computes a partial out^T (its 2 heads
pushed through the matching W_proj columns); the host sums the 8 partials.

Device layout (per core) is transpose-oriented so every matmul contracts
over the partition dim with zero on-device transposes of activations:
  qkv^T (f x t) = W_attn_slice^T.T @ x^T      [via lhsT = W_attn^T tiles]
  S^T   (s x t) = K^T.T @ Q^T                 [scores transposed]
  y^T   (d x t) = V.T @ exp(S^T)              [V transposed once on PE]
  out^T (o x t) = W_proj_slice^T.T @ y^T

All tensors fp16 (same PE rate as bf16, 8x the mantissa). The attention
inner loop processes BOTH heads per s-block so K/V stationary loads are
shared; exp runs as one fused ACT instruction over a 2-bank psum tile;
softmax denominators accumulate on DVE (elementwise adds of the exp
tiles) with a single ones-matmul per (chunk, head) at the end, keeping
the PE stream free of per-block ones-matmuls. Out-projection matmuls of
the previous chunk are sprinkled into the attention loop as PE fillers
so the tensor engine never idles (p-state ramp).
"""
import sys

sys.path.insert(0, "/opt/trn_rl_repo")
import types

import numpy as np

import concourse.bass as bass
import concourse.mybir as mybir
import concourse.tile as tile
from concourse import bacc
from concourse.bass import ts
from concourse.bass_utils import run_bass_kernel_spmd
from concourse.masks import make_identity

T, C = 4096, 2048
HS = 128
TT = 512                 # t-tile (matmul moving free dim)
NT = T // TT             # 8
NCT = C // 128           # 16 c-tiles
F = 512                  # per-core W_attn rows: 2 q heads + k + v
SCALE = 1.0 / float(np.sqrt(np.float32(HS)))

dt = mybir.dt
FP32 = dt.float32
FP16 = dt.float16
F32R = dt.float32r
AF = mybir.ActivationFunctionType
ALU = mybir.AluOpType

_cache = {}


def install_ntff_hook_shim():
    """antenv.axon_hooks is missing from this image; register the
    ctypes-based NTFF hook ourselves so trace=True works under axon."""
    if "antenv.axon_hooks" in sys.modules:
        return
    import antenv

    mod = types.ModuleType("antenv.axon_hooks")
    mod._hook = None
    mod.set_axon_ntff_profile_hook = lambda h: setattr(mod, "_hook", h)
    mod.get_axon_ntff_profile_hook = lambda: mod._hook
    sys.modules["antenv.axon_hooks"] = mod
    antenv.axon_hooks = mod
    try:
        from trn_agent_boot.trn_boot import _ntff_profile_via_ctypes

        mod.set_axon_ntff_profile_hook(
            _ntff_profile_via_ctypes("/opt/axon/libaxon_pjrt.so")
        )
    except Exception:
        pass


def build():
    nc = bacc.Bacc(
        "TRN2", target_bir_lowering=False, debug=False, enable_asserts=False
    )
    xT = nc.dram_tensor("xT", [C, T], FP16, kind="ExternalInput").ap()
    waT = nc.dram_tensor("waT", [C, F], FP16, kind="ExternalInput").ap()
    wpT = nc.dram_tensor("wpT", [2 * HS, C], FP16, kind="ExternalInput").ap()
    cos2 = nc.dram_tensor("cos2", [128, T], FP32, kind="ExternalInput").ap()
    sin2 = nc.dram_tensor("sin2", [128, T], FP32, kind="ExternalInput").ap()
    outT = nc.dram_tensor("outT", [C, T], FP16, kind="ExternalOutput").ap()

    xT_r = xT.rearrange("(a p) t -> p a t", p=128)     # [128, 16, 4096]
    waT_r = waT.rearrange("(a p) f -> p a f", p=128)   # [128, 16, 512]
    wpT_r = wpT.rearrange("(a p) o -> p a o", p=128)   # [128, 2, 2048]

    with tile.TileContext(nc) as tc:
        with (
            tc.tile_pool(name="singles", bufs=1) as singles,
            tc.tile_pool(name="xp", bufs=2) as xp,
            tc.tile_pool(name="qp", bufs=6) as qp,
            tc.tile_pool(name="kp", bufs=NT) as kp,
            tc.tile_pool(name="vp", bufs=4 * NT) as vp,
            tc.tile_pool(name="vstage", bufs=2) as vstage,
            tc.tile_pool(name="pp", bufs=6) as pp,
            tc.tile_pool(name="rtmp", bufs=4) as rtmp,
            tc.tile_pool(name="accp", bufs=2) as accp,
            tc.tile_pool(name="ysb", bufs=6) as ysb,
            tc.tile_pool(name="rbp", bufs=4) as rbp,
            tc.tile_pool(name="osb", bufs=6) as osb,
            tc.tile_pool(name="mm_ps", bufs=2, space="PSUM") as mm_ps,
            tc.tile_pool(name="s_ps", bufs=2, space="PSUM") as s_ps,
            tc.tile_pool(name="y_ps", bufs=2, space="PSUM") as y_ps,
        ):
            # ---- persistent tiles; first c-chunks as small separate
            # transfers on two queues so the first qkv matmul can start
            # within a couple of microseconds ----
            wa_sb = singles.tile([128, NCT, F], FP16)
            xt0 = xp.tile([128, NCT, TT], FP16, tag="xt")
            nc.sync.dma_start(wa_sb[:, 0:1, :], waT_r[:, 0:1, :])
            nc.scalar.dma_start(xt0[:, 0:1, :], xT_r[:, 0:1, 0:TT])
            nc.sync.dma_start(wa_sb[:, 1:2, :], waT_r[:, 1:2, :])
            nc.scalar.dma_start(xt0[:, 1:2, :], xT_r[:, 1:2, 0:TT])
            nc.sync.dma_start(wa_sb[:, 2:4, :], waT_r[:, 2:4, :])
            nc.scalar.dma_start(xt0[:, 2:4, :], xT_r[:, 2:4, 0:TT])
            for q in range(1, 4):
                nc.sync.dma_start(
                    wa_sb[:, 4 * q:4 * (q + 1), :], waT_r[:, 4 * q:4 * (q + 1), :]
                )
                nc.scalar.dma_start(
                    xt0[:, 4 * q:4 * (q + 1), :],
                    xT_r[:, 4 * q:4 * (q + 1), 0:TT],
                )
            cos_sb = singles.tile([128, T], FP32)
            sin_sb = singles.tile([128, T], FP32)
            nc.scalar.dma_start(cos_sb[:, 0:TT], cos2[:, 0:TT])
            nc.scalar.dma_start(sin_sb[:, 0:TT], sin2[:, 0:TT])
            wp_sb = singles.tile([128, 2, C], FP16)
            nc.gpsimd.dma_start(wp_sb, wpT_r)
            ident = singles.tile([128, 128], FP16)
            make_identity(nc, ident)
            ones_sb = singles.tile([128, 1], FP16)
            nc.vector.memset(ones_sb, 1.0)
            ones_row = singles.tile([1, 128], FP16)
            nc.vector.memset(ones_row, 1.0)
            # causal triangle mask: mask[p, c] = 1 iff c >= p. Every
            # diagonal s-block sees this same pattern after its suffix
            # slicing, so one tile serves all of them.
            cmask = singles.tile([128, TT], FP16)
            nc.gpsimd.memset(cmask, 1.0)
            nc.gpsimd.affine_select(
                out=cmask,
                in_=cmask,
                compare_op=ALU.is_ge,
                fill=0.0,
                base=0,
                pattern=[[1, TT]],
                channel_multiplier=-1,
            )

            q_tiles = [[None] * NT for _ in range(2)]
            k_tiles = [None] * NT
            v_tiles = [None] * (4 * NT)
            y_chunks = [None] * NT
            acc_of = [None] * NT
            yps_of = [None] * NT
            dp_of = [None] * NT
            drow_of = [None] * NT
            bc_of = [None] * NT
            rb_of = [None] * NT
            vstage_of = [None] * NT
            xts = [xt0] + [None] * (NT - 1)
            proj_pending = {}

            def _rope(cos_sl, sin_sl, src_ps, dst):
                # Rotate-half RoPE, 4 DVE ops. sin_sl rows 0:64 hold -sin
                # (sign folded on the host) so the combine is one add.
                # Two-input DVE ops need equal base partitions only when
                # BOTH inputs are SBUF; the partition-shifted operand is
                # always the PSUM one here.
                tcos = rtmp.tile([128, TT], FP16, tag="tc")
                tsin = rtmp.tile([128, TT], FP16, tag="tsn")
                nc.vector.tensor_mul(tcos, src_ps, cos_sl)
                nc.vector.tensor_mul(
                    tsin[0:64, :], src_ps[64:128, :], sin_sl[0:64, :]
                )
                nc.vector.tensor_mul(
                    tsin[64:128, :], src_ps[0:64, :], sin_sl[64:128, :]
                )
                nc.vector.tensor_add(dst, tcos, tsin)

            # ---- softmax denominator tail for chunk `pc`, emitted in
            # pieces interleaved with the NEXT chunk's qkv groups so the
            # den->drow->bc->recip chain never stalls the PE ----
            def emit_den(pc):
                dps = []
                for h in range(2):
                    dp = s_ps.tile([1, TT], FP32, tag="s")
                    nc.tensor.matmul(
                        dp, ones_sb, acc_of[pc][:, h, :], start=True, stop=True
                    )
                    dps.append(dp)
                dp_of[pc] = dps

            def emit_drow(pc, on_act=False):
                drows = []
                for h in range(2):
                    # fp16 so the bc matmul's ldweights is a fast 2-byte
                    # load (fp32r weight loads stall the PE ~0.5us)
                    drow = rbp.tile([1, TT], FP16, tag="dr")
                    if on_act:
                        # last-chunk tail: ACT is idle there, and moving
                        # these off the DVE shortens the serial
                        # den->drow->bc->recip->yt chain before the final
                        # out-projection can start
                        nc.scalar.copy(drow, dp_of[pc][h])
                    else:
                        nc.vector.tensor_copy(drow, dp_of[pc][h])
                    drows.append(drow)
                drow_of[pc] = drows

            def emit_bc(pc):
                bcs = []
                for h in range(2):
                    bc = s_ps.tile([128, TT], FP32, tag="s")
                    nc.tensor.matmul(
                        bc, ones_row, drow_of[pc][h], start=True, stop=True
                    )
                    bcs.append(bc)
                bc_of[pc] = bcs

            def emit_recip(pc):
                rbs = []
                for h in range(2):
                    rb = rbp.tile([128, TT], FP32, tag="rb")
                    nc.vector.reciprocal_approx_fast(out=rb, in_=bc_of[pc][h])
                    rbs.append(rb)
                rb_of[pc] = rbs

            def emit_yt(pc):
                yts = []
                for h in range(2):
                    yt = ysb.tile([128, TT], FP16, tag="yt")
                    nc.vector.tensor_mul(yt, yps_of[pc][h], rb_of[pc][h])
                    yts.append(yt)
                y_chunks[pc] = yts
                proj_pending[pc] = list(range(NCT))

            def emit_proj_drain(src, oi, op, force_act=False):
                ot = osb.tile([128, TT], FP16, tag="ot")
                if oi % 2 == 0 and not force_act:
                    nc.vector.tensor_copy(ot, op)
                else:
                    nc.scalar.copy(ot, op)
                nc.sync.dma_start(outT[oi * 128:(oi + 1) * 128, ts(src, TT)], ot)

            def emit_proj_oi(src, oi, defer=False, force_act=False):
                op = mm_ps.tile([128, TT], FP32, tag="mm")
                for cj in range(2):
                    nc.tensor.matmul(
                        op,
                        wp_sb[:, cj, oi * 128:(oi + 1) * 128],
                        y_chunks[src][cj],
                        start=(cj == 0),
                        stop=(cj == 1),
                    )
                if defer and oi % 2 == 0:
                    # the DVE drain would land behind this j's den-add on
                    # the in-order DVE queue and slip ~0.5-1us, stalling
                    # the next filler's psum slot; emit it at the top of
                    # the NEXT j iteration instead, ahead of that den-add
                    return (src, oi, op)
                emit_proj_drain(src, oi, op, force_act)
                return None

            for i in range(NT):
                xt = xts[i]
                # ---- QKV projection for t-chunk i, with the previous
                # chunk's denominator tail threaded between groups ----
                for f in range(4):
                    ps = mm_ps.tile([128, TT], FP32, tag="mm")
                    for ci in range(NCT):
                        nc.tensor.matmul(
                            ps,
                            wa_sb[:, ci, f * 128:(f + 1) * 128],
                            xt[:, ci, :],
                            start=(ci == 0),
                            stop=(ci == NCT - 1),
                        )
                    if f == 0 and i >= 1:
                        emit_den(i - 1)
                    if f < 2:
                        dst = qp.tile([128, TT], FP16, tag="qt")
                        q_tiles[f][i] = dst
                        _rope(cos_sb[:, ts(i, TT)], sin_sb[:, ts(i, TT)],
                              ps, dst)
                    elif f == 2:
                        # recip+yt BEFORE the K rope on the DVE queue: yt
                        # is what the proj fillers read early in the next
                        # j-loop, while k has slack until j == 4i (i >= 1)
                        if i >= 1:
                            emit_recip(i - 1)
                            emit_yt(i - 1)
                        dst = kp.tile([128, TT], FP16, tag="kt")
                        k_tiles[i] = dst
                        _rope(cos_sb[:, ts(i, TT)], sin_sb[:, ts(i, TT)],
                              ps, dst)
                    else:
                        vst = vstage.tile([128, TT], FP16, tag="vst")
                        vstage_of[i] = vst
                        # ACT is idle between chunks; keeping this copy off
                        # the DVE queue lets the j1 transpose start on time
                        nc.scalar.copy(vst, ps)
                    if f == 1 and i >= 1:
                        emit_drow(i - 1)
                        emit_bc(i - 1)

                # prefetch next chunk's x and rope tables
                if i + 1 < NT:
                    nxt = xp.tile([128, NCT, TT], FP16, tag="xt")
                    xts[i + 1] = nxt
                    for q in range(4):
                        nc.scalar.dma_start(
                            nxt[:, 4 * q:4 * (q + 1), :],
                            xT_r[:, 4 * q:4 * (q + 1), ts(i + 1, TT)],
                        )
                    nc.scalar.dma_start(
                        cos_sb[:, ts(i + 1, TT)], cos2[:, ts(i + 1, TT)]
                    )
                    nc.scalar.dma_start(
                        sin_sb[:, ts(i + 1, TT)], sin2[:, ts(i + 1, TT)]
                    )

                # ---- attention for t-chunk i, both heads fused ----
                ns = 4 * (i + 1)
                yps = [
                    y_ps.tile([128, TT], FP32, tag="y", name=f"yp{h}")
                    for h in range(2)
                ]
                yps_of[i] = yps
                acc2 = accp.tile([128, 2, TT], FP16, tag="acc")
                acc_of[i] = acc2
                fillers = proj_pending.get(i - 1, [])
                nfill = 0

                def emit_av(pj, poff, p2_sb):
                    for h in range(2):
                        nc.tensor.matmul(
                            yps[h][:, poff:], v_tiles[pj],
                            p2_sb[:, h, poff:],
                            start=(pj == 0), stop=(pj == ns - 1),
                            skip_group_check=True,
                        )

                pend = None
                pend_drain = None
                for j in range(ns):
                    diag = j >= 4 * i
                    off = (j - 4 * i) * 128 if diag else 0
                    sp2 = s_ps.tile([128, 2, TT], FP32, tag="s")
                    ksl = k_tiles[j // 4][:, (j % 4) * 128:(j % 4 + 1) * 128]
                    for h in range(2):
                        nc.tensor.matmul(
                            sp2[:, h, off:], ksl, q_tiles[h][i][:, off:],
                            start=True, stop=True, skip_group_check=True,
                        )
                    if pend_drain is not None:
                        emit_proj_drain(*pend_drain)
                        pend_drain = None
                    p2 = pp.tile([128, 2, TT], FP16, tag="p")
                    nc.scalar.activation(
                        p2[:, :, off:], sp2[:, :, off:], AF.Exp, scale=SCALE
                    )
                    if diag:
                        # zero entries with s > t inside the aligned 128-wide
                        # triangle at the start of the slice: a DVE multiply
                        # by the static mask reaches the AV/den consumers
                        # ~0.5us sooner than the gpsimd affine_select path
                        for h in range(2):
                            nc.vector.tensor_mul(
                                p2[:, h, off:], p2[:, h, off:],
                                cmask[:, 0:TT - off],
                            )
                    if j == 0:
                        nc.vector.tensor_copy(acc2, p2)
                    else:
                        nc.vector.tensor_add(
                            acc2[:, :, off:], acc2[:, :, off:], p2[:, :, off:]
                        )
                    # deferred V transposes: the new v tiles are only needed
                    # from j == 4i, and by j == 1 the DVE has drained vstage.
                    # One per j (i >= 1) so each transpose's psum slot has a
                    # full j-period of slack behind the vt drain copy.
                    if i == 0:
                        tgroup = {1: (0, 1), 2: (2, 3)}.get(j, ())
                    else:
                        tgroup = {1: (0,), 2: (1,), 3: (2,), 4: (3,)}.get(j, ())
                    if tgroup:
                        for j4 in tgroup:
                            tp = mm_ps.tile([128, 128], FP16, tag="mm")
                            nc.tensor.transpose(
                                tp,
                                vstage_of[i][:, j4 * 128:(j4 + 1) * 128],
                                ident,
                            )
                            vt = vp.tile([128, 128], FP16, tag="vt")
                            v_tiles[i * 4 + j4] = vt
                            nc.vector.tensor_copy(vt, tp)
                    # software pipeline: AV for the previous s-block is
                    # emitted AFTER this block's score matmuls so the PE
                    # never waits on exp[j] with independent work behind it
                    if pend is not None:
                        emit_av(*pend)
                    pend = (j, off, p2)
                    # out-projection fillers keep the PE p-state ramped
                    # through the exp-paced stretch of the loop
                    if fillers and j % 2 == 1 and j >= 3 and nfill < 14:
                        pend_drain = emit_proj_oi(
                            i - 1, fillers.pop(0), defer=True
                        )
                        nfill += 1
                emit_av(*pend)
                if pend_drain is not None:
                    emit_proj_drain(*pend_drain)

                if i == NT - 1:
                    emit_den(i)
                    emit_drow(i, on_act=True)
                    while fillers:
                        emit_proj_oi(i - 1, fillers.pop(0), force_act=True)
                    emit_bc(i)
                    emit_recip(i)
                    emit_yt(i)
                    for oi in range(NCT):
                        emit_proj_oi(i, oi)
                else:
                    while fillers:
                        emit_proj_oi(i - 1, fillers.pop(0), force_act=True)

    nc.compile()
    return nc


def _prep_inputs(x, cos, sin, W_attn, W_proj):
    x = np.asarray(x, dtype=np.float32)
    cos = np.asarray(cos, dtype=np.float32)
    sin = np.asarray(sin, dtype=np.float32)
    W_attn = np.asarray(W_attn, dtype=np.float32)
    W_proj = np.asarray(W_proj, dtype=np.float32)

    xT = np.ascontiguousarray(x.reshape(T, C).T).astype(np.float16)
    cos2 = np.ascontiguousarray(np.concatenate([cos.T, cos.T], axis=0))
    # sign-folded: rows 0:64 negative so RoPE's combine is a single add
    sin2 = np.ascontiguousarray(np.concatenate([-sin.T, sin.T], axis=0))

    in_maps = []
    for core in range(8):
        g = core // 2
        qoff = g * 768 + (core % 2) * 256
        rows = np.concatenate(
            [
                W_attn[qoff:qoff + 256],
                W_attn[g * 768 + 512:g * 768 + 640],
                W_attn[g * 768 + 640:g * 768 + 768],
            ],
            axis=0,
        )
        waT = np.ascontiguousarray(rows.T).astype(np.float16)
        h0 = g * 4 + (core % 2) * 2
        wpT = np.ascontiguousarray(
            W_proj[:, h0 * 128:h0 * 128 + 256].T
        ).astype(np.float16)
        in_maps.append(
            {"xT": xT, "waT": waT, "wpT": wpT, "cos2": cos2, "sin2": sin2}
        )
    return in_maps


def kernel(x, cos, sin, W_attn, W_proj, _trace=False, _trace_cores=None):
    if "nc" not in _cache:
        _cache["nc"] = build()
    nc = _cache["nc"]
    in_maps = _prep_inputs(x, cos, sin, W_attn, W_proj)
    kwargs = {}
    if _trace:
        install_ntff_hook_shim()
        kwargs = dict(trace=True, trace_cores=_trace_cores or [0])
    res = run_bass_kernel_spmd(nc, in_maps, core_ids=list(range(8)), **kwargs)
    acc = np.zeros((C, T), dtype=np.float32)
    for r in res.results:
        acc += r["outT"].astype(np.float32)
    out = np.ascontiguousarray(acc.T).reshape(1, T, C)
    _cache["last_results"] = res
    return out


# revision 47
# speedup vs baseline: 1.0070x; 1.0070x over previous
"""Trainium2 Bass kernel for nn_Block_15066745274698 (GQA attention block).

Computation (B=1, T=4096, C=2048, 16 heads x 128, 4 KV groups):
  qkv = x @ W_attn.T ; split q/k/v ; RoPE(q, k) ; causal GQA attention ;
  out = y @ W_proj.T

Sharding: head-parallel over 8 cores, 2 query heads + their KV group per
core. No collectives: each core computes a partial out^T (its 2 heads
pushed through the matching W_proj columns); the host sums the 8 partials.

Device layout (per core) is transpose-oriented so every matmul contracts
over the partition dim with zero on-device transposes of activations:
  qkv^T (f x t) = W_attn_slice^T.T @ x^T      [via lhsT = W_attn^T tiles]
  S^T   (s x t) = K^T.T @ Q^T                 [scores transposed]
  y^T   (d x t) = V.T @ exp(S^T)              [V transposed once on PE]
  out^T (o x t) = W_proj_slice^T.T @ y^T

All tensors fp16 (same PE rate as bf16, 8x the mantissa). The attention
inner loop processes BOTH heads per s-block so K/V stationary loads are
shared; exp runs as one fused ACT instruction over a 2-bank psum tile;
softmax denominators accumulate on DVE (elementwise adds of the exp
tiles) with a single ones-matmul per (chunk, head) at the end, keeping
the PE stream free of per-block ones-matmuls. Out-projection matmuls of
the previous chunk are sprinkled into the attention loop as PE fillers
so the tensor engine never idles (p-state ramp).
"""
import sys

sys.path.insert(0, "/opt/trn_rl_repo")
import types

import numpy as np

import concourse.bass as bass
import concourse.mybir as mybir
import concourse.tile as tile
from concourse import bacc
from concourse.bass import ts
from concourse.bass_utils import run_bass_kernel_spmd
from concourse.masks import make_identity

T, C = 4096, 2048
HS = 128
TT = 512                 # t-tile (matmul moving free dim)
NT = T // TT             # 8
NCT = C // 128           # 16 c-tiles
F = 512                  # per-core W_attn rows: 2 q heads + k + v
SCALE = 1.0 / float(np.sqrt(np.float32(HS)))

dt = mybir.dt
FP32 = dt.float32
FP16 = dt.float16
F32R = dt.float32r
AF = mybir.ActivationFunctionType
ALU = mybir.AluOpType

_cache = {}


def install_ntff_hook_shim():
    """antenv.axon_hooks is missing from this image; register the
    ctypes-based NTFF hook ourselves so trace=True works under axon."""
    if "antenv.axon_hooks" in sys.modules:
        return
    import antenv

    mod = types.ModuleType("antenv.axon_hooks")
    mod._hook = None
    mod.set_axon_ntff_profile_hook = lambda h: setattr(mod, "_hook", h)
    mod.get_axon_ntff_profile_hook = lambda: mod._hook
    sys.modules["antenv.axon_hooks"] = mod
    antenv.axon_hooks = mod
    try:
        from trn_agent_boot.trn_boot import _ntff_profile_via_ctypes

        mod.set_axon_ntff_profile_hook(
            _ntff_profile_via_ctypes("/opt/axon/libaxon_pjrt.so")
        )
    except Exception:
        pass


def build():
    nc = bacc.Bacc(
        "TRN2", target_bir_lowering=False, debug=False, enable_asserts=False
    )
    xT = nc.dram_tensor("xT", [C, T], FP16, kind="ExternalInput").ap()
    waT = nc.dram_tensor("waT", [C, F], FP16, kind="ExternalInput").ap()
    wpT = nc.dram_tensor("wpT", [2 * HS, C], FP16, kind="ExternalInput").ap()
    cos2 = nc.dram_tensor("cos2", [128, T], FP32, kind="ExternalInput").ap()
    sin2 = nc.dram_tensor("sin2", [128, T], FP32, kind="ExternalInput").ap()
    outT = nc.dram_tensor("outT", [C, T], FP16, kind="ExternalOutput").ap()

    xT_r = xT.rearrange("(a p) t -> p a t", p=128)     # [128, 16, 4096]
    waT_r = waT.rearrange("(a p) f -> p a f", p=128)   # [128, 16, 512]
    wpT_r = wpT.rearrange("(a p) o -> p a o", p=128)   # [128, 2, 2048]

    with tile.TileContext(nc) as tc:
        with (
            tc.tile_pool(name="singles", bufs=1) as singles,
            tc.tile_pool(name="xp", bufs=2) as xp,
            tc.tile_pool(name="qp", bufs=6) as qp,
            tc.tile_pool(name="kp", bufs=NT) as kp,
            tc.tile_pool(name="vp", bufs=4 * NT) as vp,
            tc.tile_pool(name="vstage", bufs=2) as vstage,
            tc.tile_pool(name="pp", bufs=6) as pp,
            tc.tile_pool(name="rtmp", bufs=4) as rtmp,
            tc.tile_pool(name="accp", bufs=2) as accp,
            tc.tile_pool(name="ysb", bufs=6) as ysb,
            tc.tile_pool(name="rbp", bufs=4) as rbp,
            tc.tile_pool(name="osb", bufs=6) as osb,
            tc.tile_pool(name="mm_ps", bufs=2, space="PSUM") as mm_ps,
            tc.tile_pool(name="s_ps", bufs=2, space="PSUM") as s_ps,
            tc.tile_pool(name="y_ps", bufs=2, space="PSUM") as y_ps,
        ):
            # ---- persistent tiles; first c-chunks as small separate
            # transfers on two queues so the first qkv matmul can start
            # within a couple of microseconds ----
            wa_sb = singles.tile([128, NCT, F], FP16)
            xt0 = xp.tile([128, NCT, TT], FP16, tag="xt")
            nc.sync.dma_start(wa_sb[:, 0:1, :], waT_r[:, 0:1, :])
            nc.scalar.dma_start(xt0[:, 0:1, :], xT_r[:, 0:1, 0:TT])
            nc.sync.dma_start(wa_sb[:, 1:2, :], waT_r[:, 1:2, :])
            nc.scalar.dma_start(xt0[:, 1:2, :], xT_r[:, 1:2, 0:TT])
            nc.sync.dma_start(wa_sb[:, 2:4, :], waT_r[:, 2:4, :])
            nc.scalar.dma_start(xt0[:, 2:4, :], xT_r[:, 2:4, 0:TT])
            for q in range(1, 4):
                nc.sync.dma_start(
                    wa_sb[:, 4 * q:4 * (q + 1), :], waT_r[:, 4 * q:4 * (q + 1), :]
                )
                nc.scalar.dma_start(
                    xt0[:, 4 * q:4 * (q + 1), :],
                    xT_r[:, 4 * q:4 * (q + 1), 0:TT],
                )
            cos_sb = singles.tile([128, T], FP32)
            sin_sb = singles.tile([128, T], FP32)
            nc.scalar.dma_start(cos_sb[:, 0:TT], cos2[:, 0:TT])
            nc.scalar.dma_start(sin_sb[:, 0:TT], sin2[:, 0:TT])
            wp_sb = singles.tile([128, 2, C], FP16)
            nc.gpsimd.dma_start(wp_sb, wpT_r)
            ident = singles.tile([128, 128], FP16)
            make_identity(nc, ident)
            ones_sb = singles.tile([128, 1], FP16)
            nc.vector.memset(ones_sb, 1.0)
            ones_row = singles.tile([1, 128], FP16)
            nc.vector.memset(ones_row, 1.0)
            # causal triangle mask: mask[p, c] = 1 iff c >= p. Every
            # diagonal s-block sees this same pattern after its suffix
            # slicing, so one tile serves all of them.
            cmask = singles.tile([128, TT], FP16)
            nc.gpsimd.memset(cmask, 1.0)
            nc.gpsimd.affine_select(
                out=cmask,
                in_=cmask,
                compare_op=ALU.is_ge,
                fill=0.0,
                base=0,
                pattern=[[1, TT]],
                channel_multiplier=-1,
            )

            q_tiles = [[None] * NT for _ in range(2)]
            k_tiles = [None] * NT
            v_tiles = [None] * (4 * NT)
            y_chunks = [None] * NT
            acc_of = [None] * NT
            yps_of = [None] * NT
            dp_of = [None] * NT
            drow_of = [None] * NT
            bc_of = [None] * NT
            rb_of = [None] * NT
            vstage_of = [None] * NT
            xts = [xt0] + [None] * (NT - 1)
            proj_pending = {}

            def _rope(cos_sl, sin_sl, src_ps, dst):
                # Rotate-half RoPE, 4 DVE ops. sin_sl rows 0:64 hold -sin
                # (sign folded on the host) so the combine is one add.
                # Two-input DVE ops need equal base partitions only when
                # BOTH inputs are SBUF; the partition-shifted operand is
                # always the PSUM one here.
                tcos = rtmp.tile([128, TT], FP16, tag="tc")
                tsin = rtmp.tile([128, TT], FP16, tag="tsn")
                nc.vector.tensor_mul(tcos, src_ps, cos_sl)
                nc.vector.tensor_mul(
                    tsin[0:64, :], src_ps[64:128, :], sin_sl[0:64, :]
                )
                nc.vector.tensor_mul(
                    tsin[64:128, :], src_ps[0:64, :], sin_sl[64:128, :]
                )
                nc.vector.tensor_add(dst, tcos, tsin)

            # ---- softmax denominator tail for chunk `pc`, emitted in
            # pieces interleaved with the NEXT chunk's qkv groups so the
            # den->drow->bc->recip chain never stalls the PE ----
            def emit_den(pc):
                dps = []
                for h in range(2):
                    dp = s_ps.tile([1, TT], FP32, tag="s")
                    nc.tensor.matmul(
                        dp, ones_sb, acc_of[pc][:, h, :], start=True, stop=True
                    )
                    dps.append(dp)
                dp_of[pc] = dps

            def emit_drow(pc, on_act=False):
                drows = []
                for h in range(2):
                    # fp16 so the bc matmul's ldweights is a fast 2-byte
                    # load (fp32r weight loads stall the PE ~0.5us)
                    drow = rbp.tile([1, TT], FP16, tag="dr")
                    if on_act and h == 0:
                        # last-chunk tail: both copies run in parallel
                        # (h0 on the idle ACT, h1 on DVE), shortening the
                        # serial den->drow->bc->recip->yt chain before the
                        # final out-projection can start
                        nc.scalar.copy(drow, dp_of[pc][h])
                    else:
                        nc.vector.tensor_copy(drow, dp_of[pc][h])
                    drows.append(drow)
                drow_of[pc] = drows

            def emit_bc(pc):
                bcs = []
                for h in range(2):
                    bc = s_ps.tile([128, TT], FP32, tag="s")
                    nc.tensor.matmul(
                        bc, ones_row, drow_of[pc][h], start=True, stop=True
                    )
                    bcs.append(bc)
                bc_of[pc] = bcs

            def emit_recip(pc):
                rbs = []
                for h in range(2):
                    rb = rbp.tile([128, TT], FP32, tag="rb")
                    nc.vector.reciprocal_approx_fast(out=rb, in_=bc_of[pc][h])
                    rbs.append(rb)
                rb_of[pc] = rbs

            def emit_yt(pc):
                yts = []
                for h in range(2):
                    yt = ysb.tile([128, TT], FP16, tag="yt")
                    nc.vector.tensor_mul(yt, yps_of[pc][h], rb_of[pc][h])
                    yts.append(yt)
                y_chunks[pc] = yts
                proj_pending[pc] = list(range(NCT))

            def emit_proj_drain(src, oi, op):
                ot = osb.tile([128, TT], FP16, tag="ot")
                if oi % 2 == 0:
                    nc.vector.tensor_copy(ot, op)
                else:
                    nc.scalar.copy(ot, op)
                nc.sync.dma_start(outT[oi * 128:(oi + 1) * 128, ts(src, TT)], ot)

            def emit_proj_oi(src, oi, defer=False):
                op = mm_ps.tile([128, TT], FP32, tag="mm")
                for cj in range(2):
                    nc.tensor.matmul(
                        op,
                        wp_sb[:, cj, oi * 128:(oi + 1) * 128],
                        y_chunks[src][cj],
                        start=(cj == 0),
                        stop=(cj == 1),
                    )
                if defer and oi % 2 == 0:
                    # the DVE drain would land behind this j's den-add on
                    # the in-order DVE queue and slip ~0.5-1us, stalling
                    # the next filler's psum slot; emit it at the top of
                    # the NEXT j iteration instead, ahead of that den-add
                    return (src, oi, op)
                emit_proj_drain(src, oi, op)
                return None

            for i in range(NT):
                xt = xts[i]
                # ---- QKV projection for t-chunk i, with the previous
                # chunk's denominator tail threaded between groups ----
                for f in range(4):
                    ps = mm_ps.tile([128, TT], FP32, tag="mm")
                    for ci in range(NCT):
                        nc.tensor.matmul(
                            ps,
                            wa_sb[:, ci, f * 128:(f + 1) * 128],
                            xt[:, ci, :],
                            start=(ci == 0),
                            stop=(ci == NCT - 1),
                        )
                    if f == 0 and i >= 1:
                        emit_den(i - 1)
                    if f < 2:
                        dst = qp.tile([128, TT], FP16, tag="qt")
                        q_tiles[f][i] = dst
                        _rope(cos_sb[:, ts(i, TT)], sin_sb[:, ts(i, TT)],
                              ps, dst)
                    elif f == 2:
                        # recip+yt BEFORE the K rope on the DVE queue: yt
                        # is what the proj fillers read early in the next
                        # j-loop, while k has slack until j == 4i (i >= 1)
                        if i >= 1:
                            emit_recip(i - 1)
                            emit_yt(i - 1)
                        dst = kp.tile([128, TT], FP16, tag="kt")
                        k_tiles[i] = dst
                        _rope(cos_sb[:, ts(i, TT)], sin_sb[:, ts(i, TT)],
                              ps, dst)
                    else:
                        vst = vstage.tile([128, TT], FP16, tag="vst")
                        vstage_of[i] = vst
                        # ACT is idle between chunks; keeping this copy off
                        # the DVE queue lets the j1 transpose start on time
                        nc.scalar.copy(vst, ps)
                    if f == 1 and i >= 1:
                        emit_drow(i - 1)
                        emit_bc(i - 1)

                # prefetch next chunk's x and rope tables
                if i + 1 < NT:
                    nxt = xp.tile([128, NCT, TT], FP16, tag="xt")
                    xts[i + 1] = nxt
                    for q in range(4):
                        nc.scalar.dma_start(
                            nxt[:, 4 * q:4 * (q + 1), :],
                            xT_r[:, 4 * q:4 * (q + 1), ts(i + 1, TT)],
                        )
                    nc.scalar.dma_start(
                        cos_sb[:, ts(i + 1, TT)], cos2[:, ts(i + 1, TT)]
                    )
                    nc.scalar.dma_start(
                        sin_sb[:, ts(i + 1, TT)], sin2[:, ts(i + 1, TT)]
                    )

                # ---- attention for t-chunk i, both heads fused ----
                ns = 4 * (i + 1)
                yps = [
                    y_ps.tile([128, TT], FP32, tag="y", name=f"yp{h}")
                    for h in range(2)
                ]
                yps_of[i] = yps
                acc2 = accp.tile([128, 2, TT], FP16, tag="acc")
                acc_of[i] = acc2
                fillers = proj_pending.get(i - 1, [])
                nfill = 0

                def emit_av(pj, poff, p2_sb):
                    for h in range(2):
                        nc.tensor.matmul(
                            yps[h][:, poff:], v_tiles[pj],
                            p2_sb[:, h, poff:],
                            start=(pj == 0), stop=(pj == ns - 1),
                            skip_group_check=True,
                        )

                pend = None
                pend_drain = None
                for j in range(ns):
                    diag = j >= 4 * i
                    off = (j - 4 * i) * 128 if diag else 0
                    sp2 = s_ps.tile([128, 2, TT], FP32, tag="s")
                    ksl = k_tiles[j // 4][:, (j % 4) * 128:(j % 4 + 1) * 128]
                    for h in range(2):
                        nc.tensor.matmul(
                            sp2[:, h, off:], ksl, q_tiles[h][i][:, off:],
                            start=True, stop=True, skip_group_check=True,
                        )
                    if pend_drain is not None:
                        emit_proj_drain(*pend_drain)
                        pend_drain = None
                    p2 = pp.tile([128, 2, TT], FP16, tag="p")
                    nc.scalar.activation(
                        p2[:, :, off:], sp2[:, :, off:], AF.Exp, scale=SCALE
                    )
                    if diag:
                        # zero entries with s > t inside the aligned 128-wide
                        # triangle at the start of the slice: a DVE multiply
                        # by the static mask reaches the AV/den consumers
                        # ~0.5us sooner than the gpsimd affine_select path
                        for h in range(2):
                            nc.vector.tensor_mul(
                                p2[:, h, off:], p2[:, h, off:],
                                cmask[:, 0:TT - off],
                            )
                    if j == 0:
                        nc.vector.tensor_copy(acc2, p2)
                    else:
                        nc.vector.tensor_add(
                            acc2[:, :, off:], acc2[:, :, off:], p2[:, :, off:]
                        )
                    # deferred V transposes: the new v tiles are only needed
                    # from j == 4i, and by j == 1 the DVE has drained vstage.
                    # One per j (i >= 1) so each transpose's psum slot has a
                    # full j-period of slack behind the vt drain copy.
                    if i == 0:
                        tgroup = {1: (0, 1), 2: (2, 3)}.get(j, ())
                    else:
                        tgroup = {1: (0,), 2: (1,), 3: (2,), 4: (3,)}.get(j, ())
                    if tgroup:
                        for j4 in tgroup:
                            tp = mm_ps.tile([128, 128], FP16, tag="mm")
                            nc.tensor.transpose(
                                tp,
                                vstage_of[i][:, j4 * 128:(j4 + 1) * 128],
                                ident,
                            )
                            vt = vp.tile([128, 128], FP16, tag="vt")
                            v_tiles[i * 4 + j4] = vt
                            nc.vector.tensor_copy(vt, tp)
                    # software pipeline: AV for the previous s-block is
                    # emitted AFTER this block's score matmuls so the PE
                    # never waits on exp[j] with independent work behind it
                    if pend is not None:
                        emit_av(*pend)
                    pend = (j, off, p2)
                    # out-projection fillers keep the PE p-state ramped
                    # through the exp-paced stretch of the loop
                    if fillers and j % 2 == 1 and j >= 3 and nfill < 14:
                        pend_drain = emit_proj_oi(
                            i - 1, fillers.pop(0), defer=True
                        )
                        nfill += 1
                emit_av(*pend)
                if pend_drain is not None:
                    emit_proj_drain(*pend_drain)

                if i == NT - 1:
                    emit_den(i)
                    emit_drow(i, on_act=True)
                    while fillers:
                        emit_proj_oi(i - 1, fillers.pop(0))
                    emit_bc(i)
                    # interleave recip/yt per head: yt0 (needed by the
                    # first projection matmul) lands one recip earlier
                    rbs, yts = [], []
                    for h in range(2):
                        rb = rbp.tile([128, TT], FP32, tag="rb")
                        nc.vector.reciprocal_approx_fast(
                            out=rb, in_=bc_of[i][h]
                        )
                        rbs.append(rb)
                        yt = ysb.tile([128, TT], FP16, tag="yt")
                        nc.vector.tensor_mul(yt, yps_of[i][h], rb)
                        yts.append(yt)
                    rb_of[i] = rbs
                    y_chunks[i] = yts
                    proj_pending[i] = list(range(NCT))
                    for oi in range(NCT):
                        emit_proj_oi(i, oi)
                else:
                    while fillers:
                        emit_proj_oi(i - 1, fillers.pop(0))

    nc.compile()
    return nc


def _prep_inputs(x, cos, sin, W_attn, W_proj):
    x = np.asarray(x, dtype=np.float32)
    cos = np.asarray(cos, dtype=np.float32)
    sin = np.asarray(sin, dtype=np.float32)
    W_attn = np.asarray(W_attn, dtype=np.float32)
    W_proj = np.asarray(W_proj, dtype=np.float32)

    xT = np.ascontiguousarray(x.reshape(T, C).T).astype(np.float16)
    cos2 = np.ascontiguousarray(np.concatenate([cos.T, cos.T], axis=0))
    # sign-folded: rows 0:64 negative so RoPE's combine is a single add
    sin2 = np.ascontiguousarray(np.concatenate([-sin.T, sin.T], axis=0))

    in_maps = []
    for core in range(8):
        g = core // 2
        qoff = g * 768 + (core % 2) * 256
        rows = np.concatenate(
            [
                W_attn[qoff:qoff + 256],
                W_attn[g * 768 + 512:g * 768 + 640],
                W_attn[g * 768 + 640:g * 768 + 768],
            ],
            axis=0,
        )
        waT = np.ascontiguousarray(rows.T).astype(np.float16)
        h0 = g * 4 + (core % 2) * 2
        wpT = np.ascontiguousarray(
            W_proj[:, h0 * 128:h0 * 128 + 256].T
        ).astype(np.float16)
        in_maps.append(
            {"xT": xT, "waT": waT, "wpT": wpT, "cos2": cos2, "sin2": sin2}
        )
    return in_maps


def kernel(x, cos, sin, W_attn, W_proj, _trace=False, _trace_cores=None):
    if "nc" not in _cache:
        _cache["nc"] = build()
    nc = _cache["nc"]
    in_maps = _prep_inputs(x, cos, sin, W_attn, W_proj)
    kwargs = {}
    if _trace:
        install_ntff_hook_shim()
        kwargs = dict(trace=True, trace_cores=_trace_cores or [0])
    res = run_bass_kernel_spmd(nc, in_maps, core_ids=list(range(8)), **kwargs)
    acc = np.zeros((C, T), dtype=np.float32)
    for r in res.results:
        acc += r["outT"].astype(np.float32)
    out = np.ascontiguousarray(acc.T).reshape(1, T, C)
    _cache["last_results"] = res
    return out


# revision 49
# speedup vs baseline: 1.0114x; 1.0044x over previous
"""Trainium2 Bass kernel for nn_Block_15066745274698 (GQA attention block).

Computation (B=1, T=4096, C=2048, 16 heads x 128, 4 KV groups):
  qkv = x @ W_attn.T ; split q/k/v ; RoPE(q, k) ; causal GQA attention ;
  out = y @ W_proj.T

Sharding: head-parallel over 8 cores, 2 query heads + their KV group per
core. No collectives: each core computes a partial out^T (its 2 heads
pushed through the matching W_proj columns); the host sums the 8 partials.

Device layout (per core) is transpose-oriented so every matmul contracts
over the partition dim with zero on-device transposes of activations:
  qkv^T (f x t) = W_attn_slice^T.T @ x^T      [via lhsT = W_attn^T tiles]
  S^T   (s x t) = K^T.T @ Q^T                 [scores transposed]
  y^T   (d x t) = V.T @ exp(S^T)              [V transposed once on PE]
  out^T (o x t) = W_proj_slice^T.T @ y^T

All tensors fp16 (same PE rate as bf16, 8x the mantissa). The attention
inner loop processes BOTH heads per s-block so K/V stationary loads are
shared; exp runs as one fused ACT instruction over a 2-bank psum tile;
softmax denominators accumulate on DVE (elementwise adds of the exp
tiles) with a single ones-matmul per (chunk, head) at the end, keeping
the PE stream free of per-block ones-matmuls. Out-projection matmuls of
the previous chunk are sprinkled into the attention loop as PE fillers
so the tensor engine never idles (p-state ramp).
"""
import sys

sys.path.insert(0, "/opt/trn_rl_repo")
import types

import numpy as np

import concourse.bass as bass
import concourse.mybir as mybir
import concourse.tile as tile
from concourse import bacc
from concourse.bass import ts
from concourse.bass_utils import run_bass_kernel_spmd
from concourse.masks import make_identity

T, C = 4096, 2048
HS = 128
TT = 512                 # t-tile (matmul moving free dim)
NT = T // TT             # 8
NCT = C // 128           # 16 c-tiles
F = 512                  # per-core W_attn rows: 2 q heads + k + v
SCALE = 1.0 / float(np.sqrt(np.float32(HS)))

dt = mybir.dt
FP32 = dt.float32
FP16 = dt.float16
F32R = dt.float32r
AF = mybir.ActivationFunctionType
ALU = mybir.AluOpType

_cache = {}


def install_ntff_hook_shim():
    """antenv.axon_hooks is missing from this image; register the
    ctypes-based NTFF hook ourselves so trace=True works under axon."""
    if "antenv.axon_hooks" in sys.modules:
        return
    import antenv

    mod = types.ModuleType("antenv.axon_hooks")
    mod._hook = None
    mod.set_axon_ntff_profile_hook = lambda h: setattr(mod, "_hook", h)
    mod.get_axon_ntff_profile_hook = lambda: mod._hook
    sys.modules["antenv.axon_hooks"] = mod
    antenv.axon_hooks = mod
    try:
        from trn_agent_boot.trn_boot import _ntff_profile_via_ctypes

        mod.set_axon_ntff_profile_hook(
            _ntff_profile_via_ctypes("/opt/axon/libaxon_pjrt.so")
        )
    except Exception:
        pass


def build():
    nc = bacc.Bacc(
        "TRN2", target_bir_lowering=False, debug=False, enable_asserts=False
    )
    xT = nc.dram_tensor("xT", [C, T], FP16, kind="ExternalInput").ap()
    waT = nc.dram_tensor("waT", [C, F], FP16, kind="ExternalInput").ap()
    wpT = nc.dram_tensor("wpT", [2 * HS, C], FP16, kind="ExternalInput").ap()
    cos2 = nc.dram_tensor("cos2", [128, T], FP32, kind="ExternalInput").ap()
    sin2 = nc.dram_tensor("sin2", [128, T], FP32, kind="ExternalInput").ap()
    outT = nc.dram_tensor("outT", [C, T], FP16, kind="ExternalOutput").ap()

    xT_r = xT.rearrange("(a p) t -> p a t", p=128)     # [128, 16, 4096]
    waT_r = waT.rearrange("(a p) f -> p a f", p=128)   # [128, 16, 512]
    wpT_r = wpT.rearrange("(a p) o -> p a o", p=128)   # [128, 2, 2048]

    with tile.TileContext(nc) as tc:
        with (
            tc.tile_pool(name="singles", bufs=1) as singles,
            tc.tile_pool(name="xp", bufs=2) as xp,
            tc.tile_pool(name="qp", bufs=6) as qp,
            tc.tile_pool(name="kp", bufs=NT) as kp,
            tc.tile_pool(name="vp", bufs=4 * NT) as vp,
            tc.tile_pool(name="vstage", bufs=2) as vstage,
            tc.tile_pool(name="pp", bufs=6) as pp,
            tc.tile_pool(name="rtmp", bufs=4) as rtmp,
            tc.tile_pool(name="accp", bufs=2) as accp,
            tc.tile_pool(name="ysb", bufs=6) as ysb,
            tc.tile_pool(name="rbp", bufs=4) as rbp,
            tc.tile_pool(name="osb", bufs=6) as osb,
            tc.tile_pool(name="mm_ps", bufs=2, space="PSUM") as mm_ps,
            tc.tile_pool(name="s_ps", bufs=2, space="PSUM") as s_ps,
            tc.tile_pool(name="y_ps", bufs=2, space="PSUM") as y_ps,
        ):
            # ---- persistent tiles; first c-chunks as small separate
            # transfers on two queues so the first qkv matmul can start
            # within a couple of microseconds ----
            wa_sb = singles.tile([128, NCT, F], FP16)
            xt0 = xp.tile([128, NCT, TT], FP16, tag="xt")
            nc.sync.dma_start(wa_sb[:, 0:1, :], waT_r[:, 0:1, :])
            nc.scalar.dma_start(xt0[:, 0:1, :], xT_r[:, 0:1, 0:TT])
            nc.sync.dma_start(wa_sb[:, 1:2, :], waT_r[:, 1:2, :])
            nc.scalar.dma_start(xt0[:, 1:2, :], xT_r[:, 1:2, 0:TT])
            nc.sync.dma_start(wa_sb[:, 2:4, :], waT_r[:, 2:4, :])
            nc.scalar.dma_start(xt0[:, 2:4, :], xT_r[:, 2:4, 0:TT])
            for q in range(1, 4):
                nc.sync.dma_start(
                    wa_sb[:, 4 * q:4 * (q + 1), :], waT_r[:, 4 * q:4 * (q + 1), :]
                )
                nc.scalar.dma_start(
                    xt0[:, 4 * q:4 * (q + 1), :],
                    xT_r[:, 4 * q:4 * (q + 1), 0:TT],
                )
            cos_sb = singles.tile([128, T], FP32)
            sin_sb = singles.tile([128, T], FP32)
            nc.scalar.dma_start(cos_sb[:, 0:TT], cos2[:, 0:TT])
            nc.scalar.dma_start(sin_sb[:, 0:TT], sin2[:, 0:TT])
            wp_sb = singles.tile([128, 2, C], FP16)
            nc.gpsimd.dma_start(wp_sb, wpT_r)
            ident = singles.tile([128, 128], FP16)
            make_identity(nc, ident)
            ones_sb = singles.tile([128, 1], FP16)
            nc.vector.memset(ones_sb, 1.0)
            ones_row = singles.tile([1, 128], FP16)
            nc.vector.memset(ones_row, 1.0)
            # causal triangle mask: mask[p, c] = 1 iff c >= p. Every
            # diagonal s-block sees this same pattern after its suffix
            # slicing, so one tile serves all of them.
            cmask = singles.tile([128, TT], FP16)
            nc.gpsimd.memset(cmask, 1.0)
            nc.gpsimd.affine_select(
                out=cmask,
                in_=cmask,
                compare_op=ALU.is_ge,
                fill=0.0,
                base=0,
                pattern=[[1, TT]],
                channel_multiplier=-1,
            )

            q_tiles = [[None] * NT for _ in range(2)]
            k_tiles = [None] * NT
            v_tiles = [None] * (4 * NT)
            y_chunks = [None] * NT
            acc_of = [None] * NT
            yps_of = [None] * NT
            dp_of = [None] * NT
            drow_of = [None] * NT
            bc_of = [None] * NT
            rb_of = [None] * NT
            vstage_of = [None] * NT
            xts = [xt0] + [None] * (NT - 1)
            proj_pending = {}

            def _rope(cos_sl, sin_sl, src_ps, dst):
                # Rotate-half RoPE, 4 DVE ops. sin_sl rows 0:64 hold -sin
                # (sign folded on the host) so the combine is one add.
                # Two-input DVE ops need equal base partitions only when
                # BOTH inputs are SBUF; the partition-shifted operand is
                # always the PSUM one here.
                tcos = rtmp.tile([128, TT], FP16, tag="tc")
                tsin = rtmp.tile([128, TT], FP16, tag="tsn")
                nc.vector.tensor_mul(tcos, src_ps, cos_sl)
                nc.vector.tensor_mul(
                    tsin[0:64, :], src_ps[64:128, :], sin_sl[0:64, :]
                )
                nc.vector.tensor_mul(
                    tsin[64:128, :], src_ps[0:64, :], sin_sl[64:128, :]
                )
                nc.vector.tensor_add(dst, tcos, tsin)

            # ---- softmax denominator tail for chunk `pc`, emitted in
            # pieces interleaved with the NEXT chunk's qkv groups so the
            # den->drow->bc->recip chain never stalls the PE ----
            def emit_den(pc):
                dps = []
                for h in range(2):
                    dp = s_ps.tile([1, TT], FP32, tag="s")
                    nc.tensor.matmul(
                        dp, ones_sb, acc_of[pc][:, h, :], start=True, stop=True
                    )
                    dps.append(dp)
                dp_of[pc] = dps

            def emit_drow(pc, on_act=False):
                drows = []
                for h in range(2):
                    # fp16 so the bc matmul's ldweights is a fast 2-byte
                    # load (fp32r weight loads stall the PE ~0.5us)
                    drow = rbp.tile([1, TT], FP16, tag="dr")
                    if on_act:
                        # last-chunk tail: ACT is idle there, and moving
                        # these off the DVE shortens the serial
                        # den->drow->bc->recip->yt chain before the final
                        # out-projection can start
                        nc.scalar.copy(drow, dp_of[pc][h])
                    else:
                        nc.vector.tensor_copy(drow, dp_of[pc][h])
                    drows.append(drow)
                drow_of[pc] = drows

            def emit_bc(pc):
                bcs = []
                for h in range(2):
                    bc = s_ps.tile([128, TT], FP32, tag="s")
                    nc.tensor.matmul(
                        bc, ones_row, drow_of[pc][h], start=True, stop=True
                    )
                    bcs.append(bc)
                bc_of[pc] = bcs

            def emit_recip(pc):
                rbs = []
                for h in range(2):
                    rb = rbp.tile([128, TT], FP32, tag="rb")
                    nc.vector.reciprocal_approx_fast(out=rb, in_=bc_of[pc][h])
                    rbs.append(rb)
                rb_of[pc] = rbs

            def emit_yt(pc):
                yts = []
                for h in range(2):
                    yt = ysb.tile([128, TT], FP16, tag="yt")
                    nc.vector.tensor_mul(yt, yps_of[pc][h], rb_of[pc][h])
                    yts.append(yt)
                y_chunks[pc] = yts
                proj_pending[pc] = list(range(NCT))

            def emit_proj_drain(src, oi, op):
                ot = osb.tile([128, TT], FP16, tag="ot")
                if oi % 2 == 0:
                    nc.vector.tensor_copy(ot, op)
                else:
                    nc.scalar.copy(ot, op)
                nc.sync.dma_start(outT[oi * 128:(oi + 1) * 128, ts(src, TT)], ot)

            def emit_proj_oi(src, oi, defer=False):
                op = mm_ps.tile([128, TT], FP32, tag="mm")
                for cj in range(2):
                    nc.tensor.matmul(
                        op,
                        wp_sb[:, cj, oi * 128:(oi + 1) * 128],
                        y_chunks[src][cj],
                        start=(cj == 0),
                        stop=(cj == 1),
                    )
                if defer and oi % 2 == 0:
                    # the DVE drain would land behind this j's den-add on
                    # the in-order DVE queue and slip ~0.5-1us, stalling
                    # the next filler's psum slot; emit it at the top of
                    # the NEXT j iteration instead, ahead of that den-add
                    return (src, oi, op)
                emit_proj_drain(src, oi, op)
                return None

            for i in range(NT):
                xt = xts[i]
                # ---- QKV projection for t-chunk i, with the previous
                # chunk's denominator tail threaded between groups ----
                for f in range(4):
                    ps = mm_ps.tile([128, TT], FP32, tag="mm")
                    for ci in range(NCT):
                        nc.tensor.matmul(
                            ps,
                            wa_sb[:, ci, f * 128:(f + 1) * 128],
                            xt[:, ci, :],
                            start=(ci == 0),
                            stop=(ci == NCT - 1),
                        )
                    if f == 0 and i >= 1:
                        emit_den(i - 1)
                    if f < 2:
                        dst = qp.tile([128, TT], FP16, tag="qt")
                        q_tiles[f][i] = dst
                        _rope(cos_sb[:, ts(i, TT)], sin_sb[:, ts(i, TT)],
                              ps, dst)
                    elif f == 2:
                        # recip+yt BEFORE the K rope on the DVE queue: yt
                        # is what the proj fillers read early in the next
                        # j-loop, while k has slack until j == 4i (i >= 1)
                        if i >= 1:
                            emit_recip(i - 1)
                            emit_yt(i - 1)
                        dst = kp.tile([128, TT], FP16, tag="kt")
                        k_tiles[i] = dst
                        _rope(cos_sb[:, ts(i, TT)], sin_sb[:, ts(i, TT)],
                              ps, dst)
                    else:
                        vst = vstage.tile([128, TT], FP16, tag="vst")
                        vstage_of[i] = vst
                        # ACT is idle between chunks; keeping this copy off
                        # the DVE queue lets the j1 transpose start on time
                        nc.scalar.copy(vst, ps)
                    if f == 1 and i >= 1:
                        emit_drow(i - 1)
                        emit_bc(i - 1)

                # prefetch next chunk's x and rope tables
                if i + 1 < NT:
                    nxt = xp.tile([128, NCT, TT], FP16, tag="xt")
                    xts[i + 1] = nxt
                    for q in range(4):
                        nc.scalar.dma_start(
                            nxt[:, 4 * q:4 * (q + 1), :],
                            xT_r[:, 4 * q:4 * (q + 1), ts(i + 1, TT)],
                        )
                    nc.scalar.dma_start(
                        cos_sb[:, ts(i + 1, TT)], cos2[:, ts(i + 1, TT)]
                    )
                    nc.scalar.dma_start(
                        sin_sb[:, ts(i + 1, TT)], sin2[:, ts(i + 1, TT)]
                    )

                # ---- attention for t-chunk i, both heads fused ----
                ns = 4 * (i + 1)
                yps = [
                    y_ps.tile([128, TT], FP32, tag="y", name=f"yp{h}")
                    for h in range(2)
                ]
                yps_of[i] = yps
                acc2 = accp.tile([128, 2, TT], FP16, tag="acc")
                acc_of[i] = acc2
                fillers = proj_pending.get(i - 1, [])
                nfill = 0

                def emit_av(pj, poff, p2_sb):
                    for h in range(2):
                        nc.tensor.matmul(
                            yps[h][:, poff:], v_tiles[pj],
                            p2_sb[:, h, poff:],
                            start=(pj == 0), stop=(pj == ns - 1),
                            skip_group_check=True,
                        )

                pend = None
                pend_drain = None
                for j in range(ns):
                    diag = j >= 4 * i
                    off = (j - 4 * i) * 128 if diag else 0
                    sp2 = s_ps.tile([128, 2, TT], FP32, tag="s")
                    ksl = k_tiles[j // 4][:, (j % 4) * 128:(j % 4 + 1) * 128]
                    for h in range(2):
                        nc.tensor.matmul(
                            sp2[:, h, off:], ksl, q_tiles[h][i][:, off:],
                            start=True, stop=True, skip_group_check=True,
                        )
                    if pend_drain is not None:
                        emit_proj_drain(*pend_drain)
                        pend_drain = None
                    p2 = pp.tile([128, 2, TT], FP16, tag="p")
                    nc.scalar.activation(
                        p2[:, :, off:], sp2[:, :, off:], AF.Exp, scale=SCALE
                    )
                    if diag:
                        # zero entries with s > t inside the aligned 128-wide
                        # triangle at the start of the slice: a DVE multiply
                        # by the static mask reaches the AV/den consumers
                        # ~0.5us sooner than the gpsimd affine_select path
                        for h in range(2):
                            nc.vector.tensor_mul(
                                p2[:, h, off:], p2[:, h, off:],
                                cmask[:, 0:TT - off],
                            )
                    if j == 0:
                        nc.vector.tensor_copy(acc2, p2)
                    else:
                        nc.vector.tensor_add(
                            acc2[:, :, off:], acc2[:, :, off:], p2[:, :, off:]
                        )
                    # deferred V transposes: the new v tiles are only needed
                    # from j == 4i, and by j == 1 the DVE has drained vstage.
                    # One per j (i >= 1) so each transpose's psum slot has a
                    # full j-period of slack behind the vt drain copy.
                    if i == 0:
                        tgroup = {1: (0, 1), 2: (2, 3)}.get(j, ())
                    else:
                        tgroup = {1: (0,), 2: (1,), 3: (2,), 4: (3,)}.get(j, ())
                    if tgroup:
                        for j4 in tgroup:
                            tp = mm_ps.tile([128, 128], FP16, tag="mm")
                            nc.tensor.transpose(
                                tp,
                                vstage_of[i][:, j4 * 128:(j4 + 1) * 128],
                                ident,
                            )
                            vt = vp.tile([128, 128], FP16, tag="vt")
                            v_tiles[i * 4 + j4] = vt
                            nc.vector.tensor_copy(vt, tp)
                    # software pipeline: AV for the previous s-block is
                    # emitted AFTER this block's score matmuls so the PE
                    # never waits on exp[j] with independent work behind it
                    if pend is not None:
                        emit_av(*pend)
                    pend = (j, off, p2)
                    # out-projection fillers keep the PE p-state ramped
                    # through the exp-paced stretch of the loop
                    # the last chunk's yt(i-1) lands latest on the DVE
                    # queue (largest den-tail backlog); starting its
                    # fillers at j=5 avoids a measured ~1.4us wait
                    jmin = 5 if i == NT - 1 else 3
                    if fillers and j % 2 == 1 and j >= jmin and nfill < 14:
                        pend_drain = emit_proj_oi(
                            i - 1, fillers.pop(0), defer=True
                        )
                        nfill += 1
                emit_av(*pend)
                if pend_drain is not None:
                    emit_proj_drain(*pend_drain)

                if i == NT - 1:
                    emit_den(i)
                    emit_drow(i, on_act=True)
                    while fillers:
                        emit_proj_oi(i - 1, fillers.pop(0))
                    emit_bc(i)
                    emit_recip(i)
                    emit_yt(i)
                    for oi in range(NCT):
                        emit_proj_oi(i, oi)
                else:
                    while fillers:
                        emit_proj_oi(i - 1, fillers.pop(0))

    nc.compile()
    return nc


def _prep_inputs(x, cos, sin, W_attn, W_proj):
    x = np.asarray(x, dtype=np.float32)
    cos = np.asarray(cos, dtype=np.float32)
    sin = np.asarray(sin, dtype=np.float32)
    W_attn = np.asarray(W_attn, dtype=np.float32)
    W_proj = np.asarray(W_proj, dtype=np.float32)

    xT = np.ascontiguousarray(x.reshape(T, C).T).astype(np.float16)
    cos2 = np.ascontiguousarray(np.concatenate([cos.T, cos.T], axis=0))
    # sign-folded: rows 0:64 negative so RoPE's combine is a single add
    sin2 = np.ascontiguousarray(np.concatenate([-sin.T, sin.T], axis=0))

    in_maps = []
    for core in range(8):
        g = core // 2
        qoff = g * 768 + (core % 2) * 256
        rows = np.concatenate(
            [
                W_attn[qoff:qoff + 256],
                W_attn[g * 768 + 512:g * 768 + 640],
                W_attn[g * 768 + 640:g * 768 + 768],
            ],
            axis=0,
        )
        waT = np.ascontiguousarray(rows.T).astype(np.float16)
        h0 = g * 4 + (core % 2) * 2
        wpT = np.ascontiguousarray(
            W_proj[:, h0 * 128:h0 * 128 + 256].T
        ).astype(np.float16)
        in_maps.append(
            {"xT": xT, "waT": waT, "wpT": wpT, "cos2": cos2, "sin2": sin2}
        )
    return in_maps


def kernel(x, cos, sin, W_attn, W_proj, _trace=False, _trace_cores=None):
    if "nc" not in _cache:
        _cache["nc"] = build()
    nc = _cache["nc"]
    in_maps = _prep_inputs(x, cos, sin, W_attn, W_proj)
    kwargs = {}
    if _trace:
        install_ntff_hook_shim()
        kwargs = dict(trace=True, trace_cores=_trace_cores or [0])
    res = run_bass_kernel_spmd(nc, in_maps, core_ids=list(range(8)), **kwargs)
    acc = np.zeros((C, T), dtype=np.float32)
    for r in res.results:
        acc += r["outT"].astype(np.float32)
    out = np.ascontiguousarray(acc.T).reshape(1, T, C)
    _cache["last_results"] = res
    return out
